# revision 1
# baseline (speedup 1.0000x reference)
"""Trainium2 Bass kernel for the dual cross-attention similarity module.

Math (per query q of 300, way w of 5, fp32):
  qkv from shared W; B->A attention (l=25 b-tokens over m=125 a-tokens) and
  A->B attention (l=125 a-tokens over m=25 b-tokens); outputs are negated
  squared Frobenius distances between v and softmax-reconstructions:
    qs[q,w] = -|v_b[q] - softmax(q_b k_a^T/sqrt(d)) v_a[w]|^2
    sq[q,w] = -|v_a[w] - softmax(q_a k_b^T/sqrt(d)) v_b[q]|^2

Sharding: queries split 40/core across 8 cores (300 padded to 320 with
zeros); features_a / W / constants replicated; no collectives.

Per-core design:
  * qkv tensors are produced d-major ([128 d, tokens]) straight from the
    [C, tokens]-major feature layout, so the attention matmuls need no
    transposes. Large matmuls run with float32r operands (1 cycle/row on the
    PE when the moving free dim >= 256, vs 4 for fp32).
  * exp needs no max subtraction: |logit| <= ~1.5 for this distribution.
  * B->A: scores transposed ([m, l]); unnormalized reconstruction
    R[d,l] = sum_m v_a[m,d] E[m,l]; then per-l scalars c1 = <v_b, R>,
    c2 = |R|^2, Z = sum_m E are partition-reduced on the PE into [5, TB]
    psum accumulators (way-indicator stationaries keep outputs at base
    partition 0, since matmul outputs must start at partition 0/32/64/96).
    qs = -sum_l (n_b - 2 c1/Z + c2/Z^2).
  * A->B packs 5 queries (5x25 m-tokens) on partitions and uses the Gram
    expansion |v_a - P v_b|^2 = n_a - 2<E,G>/Z + <E, Gram_b E>/Z^2 with
    block-diagonal Gram (via a blockdiag mask) and group-shifted block-ones
    stationaries accumulating Z/U/V for all 8 groups into [40, TA] psum.
"""

import numpy as np

import concourse.bass as bass
import concourse.bacc as bacc
import concourse.tile as tile
from concourse import mybir
from concourse.bass_utils import run_bass_kernel_spmd

F32 = mybir.dt.float32
F32R = mybir.dt.float32r
AL = mybir.AluOpType
AF = mybir.ActivationFunctionType
AX = mybir.AxisListType

SCALE = 0.08838834764831845  # 1/sqrt(128)
N_CORES = 8
NQ = 40          # queries per core (8 groups of 5)
LB = 25          # b tokens per query
LA = 125         # a tokens per way
NW = 5           # n_way
TB = NQ * LB     # 1000 b tokens per core
TA = NW * LA     # 625 a tokens
TAP = NW * 128   # 640: a tokens padded to 128/way for even fp32r matmuls


def _chunks(total, step=512):
    c, out = 0, []
    while c < total:
        out.append((c, min(step, total - c)))
        c += step
    return out


def _f(ap):
    """fp32 view for odd-width matmuls (fp32r requires even moving dims)."""
    return ap.bitcast(F32)


def build_nc():
    nc = bacc.Bacc("TRN2", target_bir_lowering=False, debug=False)

    fa_d = nc.dram_tensor("fa", [640, TAP], F32R, kind="ExternalInput")
    fb_d = nc.dram_tensor("fb", [640, TB], F32R, kind="ExternalInput")
    wt_d = nc.dram_tensor("wt", [640, 384], F32R, kind="ExternalInput")
    ident_d = nc.dram_tensor("ident", [128, 128], F32R, kind="ExternalInput")
    ones128_d = nc.dram_tensor("ones128", [128, 1], F32R, kind="ExternalInput")
    ones5r_d = nc.dram_tensor("ones5r", [1, 5], F32R, kind="ExternalInput")
    ow5_d = nc.dram_tensor("ow5", [125, 5, 5], F32R, kind="ExternalInput")
    ow128_d = nc.dram_tensor("ow128", [128, 5, 5], F32R, kind="ExternalInput")
    bo40_d = nc.dram_tensor("bo40", [125, 8, 40], F32R, kind="ExternalInput")
    bdm_d = nc.dram_tensor("bdm", [125, 125], F32, kind="ExternalInput")
    half125_d = nc.dram_tensor("half125", [1, 125], F32R, kind="ExternalInput")
    sq_d = nc.dram_tensor("sq", [NQ, NW], F32, kind="ExternalOutput")
    qs_d = nc.dram_tensor("qs", [NW, NQ], F32, kind="ExternalOutput")

    with tile.TileContext(nc) as tc:
        with (
            tc.tile_pool(name="const", bufs=1) as const,
            tc.tile_pool(name="feat", bufs=1) as feat,
            tc.tile_pool(name="persist", bufs=1) as persist,
            tc.tile_pool(name="ew", bufs=1) as ew,
            tc.tile_pool(name="work", bufs=2) as work,
        ):
            # ---- loads ----
            ident = const.tile([128, 128], F32R)
            nc.sync.dma_start(out=ident, in_=ident_d[:])
            ones128 = const.tile([128, 1], F32R)
            nc.sync.dma_start(out=ones128, in_=ones128_d[:])
            ones5r = const.tile([1, 5], F32R)
            nc.sync.dma_start(out=ones5r, in_=ones5r_d[:])
            ow5 = const.tile([125, 5, 5], F32R)
            nc.sync.dma_start(out=ow5, in_=ow5_d[:])
            ow128 = const.tile([128, 5, 5], F32R)
            nc.sync.dma_start(out=ow128, in_=ow128_d[:])
            bo40 = const.tile([125, 8, 40], F32R)
            nc.sync.dma_start(out=bo40, in_=bo40_d[:])
            bdm = const.tile([125, 125], F32)
            nc.sync.dma_start(out=bdm, in_=bdm_d[:])
            half125 = const.tile([1, 125], F32R)
            nc.sync.dma_start(out=half125, in_=half125_d[:])

            wt = feat.tile([128, 5, 384], F32R)
            wt_r = wt_d.rearrange("(cb c) e -> c cb e", c=128)
            fa = feat.tile([128, 5, TAP], F32R)
            fa_r = fa_d.rearrange("(cb c) t -> c cb t", c=128)
            fb = feat.tile([128, 5, TB], F32R)
            fb_r = fb_d.rearrange("(cb c) t -> c cb t", c=128)
            for cb in range(5):
                nc.sync.dma_start(out=wt[:, cb, :], in_=wt_r[:, cb, :])
            for cb in range(5):
                nc.sync.dma_start(out=fa[:, cb, :], in_=fa_r[:, cb, :])
            for cb in range(5):
                nc.sync.dma_start(out=fb[:, cb, 0:500], in_=fb_r[:, cb, 0:500])
            for cb in range(5):
                nc.sync.dma_start(out=fb[:, cb, 500:TB], in_=fb_r[:, cb, 500:TB])

            # warm the ACT table set (exp_and_others holds exp/copy/square)
            warm = work.tile([1, 1], F32, tag="warm")
            nc.scalar.activation(out=warm, in_=ones128[0:1, 0:1], func=AF.Exp)

            # ---- phase A: qkv (d-major), token-major v_a, n_a, n_b ----
            psA_cm = tc.tile_pool(name="psA", bufs=1, space="PSUM")
            psA = psA_cm.__enter__()
            qkv_b = persist.tile([128, 3, TB], F32R)  # q/k/v for b tokens
            qkv_a = persist.tile([128, 3, TAP], F32R)
            for src, dst, total in ((fa, qkv_a, TAP), (fb, qkv_b, TB)):
                for e in range(3):
                    for c0, cn in _chunks(total, 320 if total == TAP else 500):
                        pqkv = psA.tile([128, 512], F32, tag="pqkv", bufs=3)
                        for cb in range(5):
                            nc.tensor.matmul(
                                pqkv[:, :cn],
                                (wt[:, cb, e * 128:(e + 1) * 128]),
                                (src[:, cb, c0:c0 + cn]),
                                start=(cb == 0),
                                stop=(cb == 4),
                            )
                        nc.scalar.copy(out=dst[:, e, c0:c0 + cn], in_=pqkv[:, :cn])
            qT_b, kT_b, vT_b = (qkv_b[:, i, :] for i in range(3))
            qT_a, kT_a, vT_a = (qkv_a[:, i, :] for i in range(3))

            vA_tok = persist.tile([125, 5, 128], F32R)
            for w in range(5):
                ptp = psA.tile([125, 128], F32, tag="ptp", bufs=2)
                nc.tensor.transpose(ptp, _f(vT_a[:, w * 128:w * 128 + 125]),
                                    _f(ident))
                nc.scalar.copy(out=vA_tok[:, w, :], in_=ptp)

            # n_a[l] = |v_a[w][l]|^2 ; na_rep = 0.5*n_a on 125 partitions
            sqa = work.tile([128, TAP], F32R, tag="sqa")
            nc.scalar.activation(out=sqa, in_=vT_a, func=AF.Square)
            pna = psA.tile([1, TAP], F32, tag="pwide", bufs=1,
                           padded_shape=[128, TB])
            for c0, cn in _chunks(TAP):
                nc.tensor.matmul(pna[:, c0:c0 + cn], (ones128),
                                 (sqa[:, c0:c0 + cn]), start=True, stop=True)
            na_sb = persist.tile([1, TAP], F32R)
            nc.scalar.copy(out=na_sb, in_=pna)
            na_rep = persist.tile([125, TAP], F32R)
            pnar = psA.tile([125, TAP], F32, tag="pwide", bufs=1,
                            padded_shape=[128, TB])
            for c0, cn in _chunks(TAP):
                nc.tensor.matmul(pnar[:, c0:c0 + cn], (half125),
                                 (na_sb[:, c0:c0 + cn]), start=True, stop=True)
            nc.scalar.copy(out=na_rep, in_=pnar)

            # n_b[l] = |v_b[q][l]|^2 replicated to [5, TB]
            sqb = work.tile([128, TB], F32R, tag="sqb")
            nc.scalar.activation(out=sqb, in_=vT_b, func=AF.Square)
            pnb = psA.tile([1, TB], F32, tag="pwide", bufs=1,
                           padded_shape=[128, TB])
            for c0, cn in _chunks(TB):
                nc.tensor.matmul(pnb[:, c0:c0 + cn], (ones128),
                                 (sqb[:, c0:c0 + cn]), start=True, stop=True)
            nb_sb = persist.tile([1, TB], F32R)
            nc.scalar.copy(out=nb_sb, in_=pnb)
            nbrep = persist.tile([5, TB], F32)
            pnbr = psA.tile([5, TB], F32, tag="pwide", bufs=1,
                            padded_shape=[128, TB])
            for c0, cn in _chunks(TB):
                nc.tensor.matmul(pnbr[:, c0:c0 + cn], (ones5r),
                                 (nb_sb[:, c0:c0 + cn]), start=True, stop=True)
            nc.scalar.copy(out=nbrep, in_=pnbr)

            bds = persist.tile([125, 8, 125], F32R)
            for g in range(8):
                pgram = psA.tile([125, 125], F32, tag="pgram", bufs=1)
                nc.tensor.matmul(pgram, _f(vT_b[:, g * 125:(g + 1) * 125]),
                                 _f(vT_b[:, g * 125:(g + 1) * 125]),
                                 start=True, stop=True)
                # zero the off-diagonal query blocks via a blockdiag mask
                nc.vector.tensor_mul(bds[:, g, :], pgram, bdm)

            psA_cm.__exit__(None, None, None)

            # ---- phase B: B attends A ----
            psB_cm = tc.tile_pool(name="psB", bufs=1, space="PSUM")
            psB = psB_cm.__enter__()
            qs_sb = work.tile([5, NQ], F32, tag="qs_sb")
            for ci, (c0, cn) in enumerate(_chunks(TB, 500)):
                ZC = psB.tile([5, 500], F32, tag="ZC", bufs=1)
                C1 = psB.tile([5, 500], F32, tag="C1", bufs=1)
                C2 = psB.tile([5, 500], F32, tag="C2", bufs=1)
                for w in range(5):
                    sba = psB.tile([125, 500], F32, tag="sba", bufs=2)
                    nc.tensor.matmul(sba[:, :cn],
                                     kT_a[:, w * 128:w * 128 + 125],
                                     qT_b[:, c0:c0 + cn],
                                     start=True, stop=True)
                    e_w = work.tile([125, 500], F32R, tag="e_w", bufs=3)
                    nc.scalar.activation(out=e_w[:, :cn], in_=sba[:, :cn],
                                         func=AF.Exp, scale=SCALE)
                    nc.tensor.matmul(ZC[:, :cn], (ow5[:, w, :]),
                                     (e_w[0:125, :cn]),
                                     start=(w == 0), stop=(w == 4))
                    rp = psB.tile([128, 500], F32, tag="rp", bufs=3)
                    nc.tensor.matmul(rp[:, :cn], (vA_tok[:, w, :]),
                                     (e_w[:, :cn]), start=True, stop=True)
                    c1sb = work.tile([128, 500], F32R, tag="c1sb", bufs=3)
                    nc.vector.tensor_mul(c1sb[:, :cn], rp[:, :cn],
                                         vT_b[:, c0:c0 + cn])
                    nc.tensor.matmul(C1[:, :cn], (ow128[:, w, :]),
                                     (c1sb[:, :cn]),
                                     start=(w == 0), stop=(w == 4))
                    c2sb = work.tile([128, 500], F32R, tag="c2sb", bufs=3)
                    nc.scalar.activation(out=c2sb[:, :cn], in_=rp[:, :cn],
                                         func=AF.Square)
                    nc.tensor.matmul(C2[:, :cn], (ow128[:, w, :]),
                                     (c2sb[:, :cn]),
                                     start=(w == 0), stop=(w == 4))

                # qs = -sum_{l in q} (n_b - 2 c1 r + c2 r^2),  r = 1/Z
                rba = work.tile([5, 500], F32, tag="rba")
                nc.vector.reciprocal(out=rba[:, :cn], in_=ZC[0:5, :cn])
                t1 = work.tile([5, 500], F32, tag="t1")
                nc.vector.tensor_mul(t1[:, :cn], C2[0:5, :cn], rba[:, :cn])
                t2 = work.tile([5, 500], F32, tag="t2")
                nc.vector.scalar_tensor_tensor(out=t2[:, :cn], in0=C1[0:5, :cn],
                                               scalar=-2.0, in1=t1[:, :cn],
                                               op0=AL.mult, op1=AL.add)
                t3 = work.tile([5, 500], F32, tag="t3")
                nc.vector.tensor_mul(t3[:, :cn], t2[:, :cn], rba[:, :cn])
                fba = work.tile([5, 500], F32, tag="fba")
                nc.vector.tensor_add(fba[:, :cn], t3[:, :cn],
                                     nbrep[:, c0:c0 + cn])
                nq_c = cn // LB
                nc.vector.tensor_reduce(
                    out=qs_sb[:, ci * 20:ci * 20 + nq_c],
                    in_=fba[:, :cn].rearrange("p (q l) -> p q l", q=nq_c),
                    op=AL.add, axis=AX.X, negate=True)
            nc.sync.dma_start(out=qs_d[:], in_=qs_sb)
            psB_cm.__exit__(None, None, None)

            # ---- phase C: A attends B (query-packed Gram expansion) ----
            psC_cm = tc.tile_pool(name="psC", bufs=1, space="PSUM")
            psC = psC_cm.__enter__()

            # l chunks of 320 (2.5 ways); way w spans cols [128w, 128w+125).
            sq_sb = work.tile([NQ, NW], F32, tag="sq_sb")
            w2part = work.tile([NQ, 2], F32, tag="w2part")
            for ci, (c0, cn) in enumerate(_chunks(TAP, 320)):
                Zp = psC.tile([NQ, 320], F32, tag="Zp", bufs=1)
                Up = psC.tile([NQ, 320], F32, tag="Up", bufs=1)
                Vp = psC.tile([NQ, 320], F32, tag="Vp", bufs=1)
                for g in range(8):
                    gsl = slice(g * 125, (g + 1) * 125)
                    sab = psC.tile([125, 320], F32, tag="pab", bufs=4)
                    nc.tensor.matmul(sab[:, :cn], kT_b[:, gsl],
                                     qT_a[:, c0:c0 + cn], start=True, stop=True)
                    eg = work.tile([125, 320], F32R, tag="eg", bufs=3)
                    nc.scalar.activation(out=eg[:, :cn], in_=sab[:, :cn],
                                         func=AF.Exp, scale=SCALE)

                    gab = psC.tile([125, 320], F32, tag="pab", bufs=4)
                    nc.tensor.matmul(gab[:, :cn], vT_b[:, gsl],
                                     vT_a[:, c0:c0 + cn], start=True, stop=True)
                    gpp = work.tile([125, 320], F32R, tag="gpp", bufs=3)
                    nc.vector.tensor_sub(gpp[:, :cn], gab[:, :cn],
                                         na_rep[:, c0:c0 + cn])

                    aeb = psC.tile([125, 320], F32, tag="pab", bufs=4)
                    nc.tensor.matmul(aeb[:, :cn], bds[:, g, :],
                                     eg[:, :cn], start=True, stop=True)

                    egp = work.tile([125, 320], F32R, tag="egp", bufs=3)
                    nc.gpsimd.tensor_mul(egp[:, :cn], eg[:, :cn], gpp[:, :cn])
                    eab = work.tile([125, 320], F32R, tag="eab", bufs=3)
                    nc.vector.tensor_mul(eab[:, :cn], eg[:, :cn], aeb[:, :cn])

                    for rhs, dst in ((eg, Zp), (egp, Up), (eab, Vp)):
                        nc.tensor.matmul(dst[:, :cn], bo40[:, g, :],
                                         rhs[:, :cn],
                                         start=(g == 0), stop=(g == 7))

                # sq = sum_l (2*U*r - V*r^2) with r = 1/Z; way 2 straddles the
                # chunk boundary (cols 256..381 global), summed via w2part.
                rab = work.tile([NQ, 320], F32, tag="rab")
                nc.vector.reciprocal(out=rab[:, :cn], in_=Zp[0:NQ, :cn])
                u1 = work.tile([NQ, 320], F32, tag="u1")
                nc.vector.tensor_mul(u1[:, :cn], Vp[0:NQ, :cn], rab[:, :cn])
                u2 = work.tile([NQ, 320], F32, tag="u2")
                nc.vector.scalar_tensor_tensor(out=u2[:, :cn], in0=Up[0:NQ, :cn],
                                               scalar=2.0, in1=u1[:, :cn],
                                               op0=AL.mult, op1=AL.subtract)
                f2 = work.tile([NQ, 320], F32, tag="f2")
                nc.vector.tensor_mul(f2[:, :cn], u2[:, :cn], rab[:, :cn])
                if ci == 0:
                    # ways 0,1 full; way 2 cols 256..320 partial
                    nc.vector.tensor_reduce(
                        out=sq_sb[:, 0:2],
                        in_=f2[:, 0:256].rearrange("p (w l) -> p w l", w=2)[:, :, 0:125],
                        op=AL.add, axis=AX.X)
                    nc.vector.tensor_reduce(
                        out=w2part[:, 0:1],
                        in_=f2[:, 256:320].rearrange("p (o l) -> p o l", o=1),
                        op=AL.add, axis=AX.X)
                else:
                    # way 2 cols 0..61 local (global 320..381); ways 3,4 full
                    nc.vector.tensor_reduce(
                        out=w2part[:, 1:2],
                        in_=f2[:, 0:61].rearrange("p (o l) -> p o l", o=1),
                        op=AL.add, axis=AX.X)
                    nc.vector.tensor_reduce(
                        out=sq_sb[:, 3:5],
                        in_=f2[:, 64:320].rearrange("p (w l) -> p w l", w=2)[:, :, 0:125],
                        op=AL.add, axis=AX.X)
            nc.vector.tensor_add(sq_sb[:, 2:3], w2part[:, 0:1], w2part[:, 1:2])
            nc.sync.dma_start(out=sq_d[:], in_=sq_sb)
            psC_cm.__exit__(None, None, None)

    nc.compile()
    return nc


_CACHE = {}
_last_in_maps = None


def _get_nc():
    if "nc" not in _CACHE:
        _CACHE["nc"] = build_nc()
    return _CACHE["nc"]


def _consts():
    ident = np.eye(128, dtype=np.float32)
    ones128 = np.ones((128, 1), np.float32)
    ones5r = np.ones((1, 5), np.float32)
    ow5 = np.zeros((125, 5, 5), np.float32)
    ow128 = np.zeros((128, 5, 5), np.float32)
    for w in range(5):
        ow5[:, w, w] = 1.0
        ow128[:, w, w] = 1.0
    bo125 = np.kron(np.eye(5, dtype=np.float32), np.ones((25, 1), np.float32))
    bo40 = np.zeros((125, 8, 40), np.float32)
    for g in range(8):
        bo40[:, g, 5 * g:5 * g + 5] = bo125
    bdm = np.kron(np.eye(5, dtype=np.float32), np.ones((25, 25), np.float32))
    half125 = np.full((1, 125), 0.5, np.float32)
    return dict(ident=ident, ones128=ones128, ones5r=ones5r, ow5=ow5,
                ow128=ow128, bo40=bo40, bdm=bdm, half125=half125)


def kernel(features_a, features_b, W):
    global _last_in_maps
    features_a = np.asarray(features_a, np.float32)
    features_b = np.asarray(features_b, np.float32)
    W = np.asarray(W, np.float32)

    nq_total = features_b.shape[0]
    fbp = np.zeros((N_CORES * NQ, 640, LB), np.float32)
    fbp[:nq_total] = features_b
    fb_t = np.ascontiguousarray(fbp.transpose(1, 0, 2))  # [640, 320, 25]
    fa_pad = np.zeros((640, NW, 128), np.float32)
    fa_pad[:, :, :LA] = features_a.transpose(1, 0, 2)
    fa_t = np.ascontiguousarray(fa_pad.reshape(640, TAP))
    wt = np.ascontiguousarray(W.T)
    consts = _consts()

    in_maps = []
    for c in range(N_CORES):
        m = {
            "fa": fa_t,
            "fb": np.ascontiguousarray(
                fb_t[:, c * NQ:(c + 1) * NQ, :]).reshape(640, TB),
            "wt": wt,
        }
        m.update(consts)
        in_maps.append(m)

    _last_in_maps = in_maps
    nc = _get_nc()
    res = run_bass_kernel_spmd(nc, in_maps, core_ids=list(range(N_CORES)))

    sq = np.zeros((N_CORES * NQ, NW), np.float32)
    qs = np.zeros((N_CORES * NQ, NW), np.float32)
    for c in range(N_CORES):
        sq[c * NQ:(c + 1) * NQ] = res.results[c]["sq"]
        qs[c * NQ:(c + 1) * NQ] = res.results[c]["qs"].T
    return sq[:nq_total], qs[:nq_total]

